# revision 55
# baseline (speedup 1.0000x reference)
"""Trainium2 Bass kernel for nn_Attention_50173807952647.

GQA attention block: qkv projections + partial interleaved RoPE + softmax
attention + output projection, fp32 inputs/outputs.

Sharding: 8 cores; core d owns kv-head d and query heads {2d, 2d+1} for all
4 batches (head/tensor parallel per the GQA grouping). Each core computes a
partial output (its heads' contribution through Wo); host sums partials + bias.

v2 layout strategy (per core), all matmuls in bf16 (PSUM accumulation fp32):
  x^T pre-transposed + cast to bf16 on the HOST -> DMA'd straight into SBUF
  (no on-chip transposes of x at all).
  q^T = Wq_d.T x [hd,t]   matmul, hd = 2 heads x 64 stacked on partitions
  kv^T            [k;v]   matmul, k rows 0:64, v^T rows 64:128
  RoPE applied in transposed layout on DVE (sign-folded sin, pair-swap
  stream_shuffle), output cast to bf16.
  k duplicated onto partitions 64:128 so scores run as TWO CONCURRENT
  row-tiled (64x128 PE tile mode) K=64 matmuls per key tile:
    head0: kdup[0:64].T @ qT[0:64]  -> sps[:, 0:512]    (PE tile (0,0))
    head1: kdup[64:128].T @ qT[64:128] -> sps[:,512:]   (PE tile (64,0))
  P = exp(S^T * scale) on ACT straight PSUM->SBUF (bf16 out; no max
  subtraction: scores are ~N(0,1) after the 1/8 scale)
  O^T[d,q], denom = [v|1].T @ P accumulated in PSUM over k tiles
  normalize via reciprocal_approx + gpsimd partition_broadcast on DVE
  out_partial[t,e] = O^T_allheads.T @ Wo_d, DMA'd to DRAM direct from PSUM.
"""

import sys

import numpy as np

HEADS = 16
KV_HEADS = 8
DIM_HEAD = 64
ROT_DIM = 32
SCALE = DIM_HEAD ** -0.5
B, N, DIM = 4, 2048, 1024
N_CORES = 8
T = B * N  # 8192 tokens
CHUNK = 512  # projection chunk (tokens)
QB = 512  # attention query block
ROWTILE = True  # scores as 2 concurrent K=64 row-tiled matmuls

_BUILT = {}


def _ensure_path():
    for p in ("/opt/trn_rl_repo",):
        if p not in sys.path:
            sys.path.insert(0, p)


def _rope_tables():
    """cos/sin tables [128, N] for the transposed [hd, t] layout.

    Row r (hd index within a core's 128 q-rows): head-local d = r % 64.
    d < ROT_DIM: cos(t * inv_freq[d//2]); sin with rotate-half sign folded
    (-sin on even d, +sin on odd d). Elsewhere cos=1, sin=0 so a single
    full-width mul+add applies RoPE only where it belongs.
    """
    inv_freq = 1.0 / (10000.0 ** (np.arange(0, ROT_DIM, 2, dtype=np.float64) / ROT_DIM))
    t = np.arange(N, dtype=np.float64)
    freqs = t[None, :] * inv_freq[:, None]  # [16, N]
    cos = np.ones((128, N), dtype=np.float64)
    sin = np.zeros((128, N), dtype=np.float64)
    for r in range(128):
        d = r % 64
        if d < ROT_DIM:
            f = freqs[d // 2]
            cos[r] = np.cos(f)
            sin[r] = (-1.0 if d % 2 == 0 else 1.0) * np.sin(f)
    return cos.astype(np.float32), sin.astype(np.float32)


def _build(debug=False):
    key = ("nc", debug, ROWTILE)
    if key in _BUILT:
        return _BUILT[key]
    _ensure_path()
    import concourse.bass as bass  # noqa: F401
    import concourse.mybir as mybir
    import concourse.tile as tile
    from concourse import bacc
    from concourse.masks import make_identity

    dt = mybir.dt
    f32, bf16 = dt.float32, dt.bfloat16
    AF = mybir.ActivationFunctionType
    OP = mybir.AluOpType

    nc = bacc.Bacc("TRN2", target_bir_lowering=False, debug=False)

    # host-preprocessed inputs (bf16, pre-transposed / pre-sliced per core)
    xt_in = nc.dram_tensor("xt", [DIM, T], bf16, kind="ExternalInput").ap()
    wq_in = nc.dram_tensor("wq", [128, DIM], bf16, kind="ExternalInput").ap()
    wkv_in = nc.dram_tensor("wkv", [128, DIM], bf16, kind="ExternalInput").ap()
    wo_in = nc.dram_tensor("wo", [128, DIM], bf16, kind="ExternalInput").ap()
    cos_in = nc.dram_tensor("cos_t", [128, N], f32, kind="ExternalInput").ap()
    sin_in = nc.dram_tensor("sin_t", [128, N], f32, kind="ExternalInput").ap()
    out_d = nc.dram_tensor("out", [T, DIM], bf16, kind="ExternalOutput").ap()
    if debug:
        dbg_qT = nc.dram_tensor("dbg_qT", [128, N], bf16, kind="ExternalOutput").ap()
        dbg_kdup = nc.dram_tensor("dbg_kdup", [128, N], bf16, kind="ExternalOutput").ap()
        dbg_v = nc.dram_tensor("dbg_v", [128, (N // 128) * 128], bf16, kind="ExternalOutput").ap()
        dbg_e = nc.dram_tensor("dbg_e", [128, 1024], bf16, kind="ExternalOutput").ap()
        dbg_ou = nc.dram_tensor("dbg_ou", [64, 1024], f32, kind="ExternalOutput").ap()
        dbg_rec = nc.dram_tensor("dbg_rec", [1, 1024], f32, kind="ExternalOutput").ap()

    NCH = N // CHUNK  # chunks per batch
    NQB = N // QB  # q blocks per batch
    NKT = N // 128  # key tiles per batch
    pair_mask = []
    for i in range(16):
        pair_mask += [2 * i + 1, 2 * i]

    with tile.TileContext(nc) as tc:
        with (
            tc.tile_pool(name="const", bufs=1) as constp,
            tc.tile_pool(name="perbatch", bufs=3) as batchp,
            tc.tile_pool(name="xt", bufs=2) as xtp,
            tc.tile_pool(name="rope", bufs=5) as ropep,
            tc.tile_pool(name="sm", bufs=2) as smp,
            tc.tile_pool(name="exp", bufs=8) as expp,
            tc.tile_pool(name="osb", bufs=3) as osbp,
            tc.tile_pool(name="psS", bufs=2, space="PSUM") as psS,
            tc.tile_pool(name="psO", bufs=1, space="PSUM") as psO,
            tc.tile_pool(name="psP", bufs=2, space="PSUM") as psP,
        ):
            ident = constp.tile([64, 64], bf16)
            make_identity(nc, ident[:])
            wq_sb = constp.tile([128, DIM], bf16, tag="wq")
            wkv_sb = constp.tile([128, DIM], bf16, tag="wkv")
            wo_sb = constp.tile([128, DIM], bf16, tag="wo")
            nc.sync.dma_start(wq_sb[:], wq_in[:])
            nc.sync.dma_start(wkv_sb[:], wkv_in[:])
            nc.sync.dma_start(wo_sb[:], wo_in[:])
            cos_sb = constp.tile([128, N], f32, tag="cos")
            sin_sb = constp.tile([128, N], f32, tag="sin")
            nc.sync.dma_start(cos_sb[:], cos_in[:])
            nc.sync.dma_start(sin_sb[:], sin_in[:])

            def load_xt(b, wide=False):
                # chunk-major DMA order matches the projection's consumption
                # order so the first proj matmul only waits on a small slice
                xT = xtp.tile([128, 8 * N], bf16, tag="xT")
                W = 1024 if wide else CHUNK
                for c in range(N // W):
                    for et in range(8):
                        nc.sync.dma_start(
                            xT[:, et * N + c * W: et * N + (c + 1) * W],
                            xt_in[et * 128:(et + 1) * 128,
                                  b * N + c * W: b * N + (c + 1) * W])
                return xT

            def rope_epilogue(c, tiles, qps_ap, kvps_ap):
                """RoPE + k-dup + v-transpose for one 512-token chunk c.
                qps_ap/kvps_ap: [128, 512] PSUM views (q / kv projections)."""
                qT, kdup, v_sb = tiles
                cs = slice(c * CHUNK, (c + 1) * CHUNK)
                shq = ropep.tile([128, CHUNK], f32, tag="rope")
                nc.vector.stream_shuffle(shq[:], qps_ap, pair_mask)
                t1q = ropep.tile([128, CHUNK], f32, tag="rope")
                nc.vector.tensor_tensor(t1q[:], qps_ap, cos_sb[:, cs], op=OP.mult)
                t2q = ropep.tile([128, CHUNK], f32, tag="rope")
                nc.vector.tensor_tensor(t2q[:], shq[:], sin_sb[:, cs], op=OP.mult)
                nc.vector.tensor_tensor(qT[:, cs], t1q[:], t2q[:], op=OP.add)
                yield
                # k -> kdup rows 0:64 (rows 32:64 unrotated)
                shk = ropep.tile([32, CHUNK], f32, tag="ropek")
                nc.vector.stream_shuffle(shk[:], kvps_ap[0:32, :], pair_mask)
                t1k = ropep.tile([64, CHUNK], f32, tag="ropek")
                nc.vector.tensor_tensor(t1k[:], kvps_ap[0:64, :],
                                        cos_sb[0:64, cs], op=OP.mult)
                t2k = ropep.tile([32, CHUNK], f32, tag="ropek")
                nc.vector.tensor_tensor(t2k[:], shk[:], sin_sb[0:32, cs],
                                        op=OP.mult)
                nc.vector.tensor_tensor(kdup[0:32, cs], t1k[0:32, :], t2k[:],
                                        op=OP.add)
                nc.vector.tensor_copy(kdup[32:64, cs], t1k[32:64, :])
                # duplicate k onto partitions 64:128 for the row-tiled scores
                nc.sync.dma_start(kdup[64:128, cs], kdup[0:64, cs])
                yield
                # v fixup: transpose [vdim, t] -> [t, vdim] per 128-token tile
                vts = ropep.tile([64, CHUNK], bf16, tag="ropev")
                nc.vector.tensor_copy(vts[:], kvps_ap[64:128, :])
                vtp_f = psP.tile([128, 512], f32, tag="ps_small")
                vtp = vtp_f[:].bitcast(bf16)[:, 0:512]
                for st in range(4):
                    nc.tensor.transpose(vtp[:, st * 128: st * 128 + 64],
                                        vts[:, st * 128:(st + 1) * 128],
                                        ident[:])
                yield
                vdst = v_sb[:].rearrange("p (kt c) -> p kt c", c=128)[
                    :, c * 4:(c + 1) * 4, 0:64]
                vsrc = vtp.rearrange("p (st c) -> p st c", c=128)[:, :, 0:64]
                nc.vector.tensor_copy(vdst, vsrc)
                yield

            def proj_qkv(b, c, tiles, xT):
                """Generator: projection chunk c (512 tokens) of batch b.

                Yields after each pair of matmuls so the caller can
                interleave with attention work on the PE queue.
                """
                qT, kdup, v_sb = tiles
                cs = slice(c * CHUNK, (c + 1) * CHUNK)
                qps = psP.tile([128, 512], f32, tag="ps_small")
                for et in range(8):
                    nc.tensor.matmul(qps[:],
                                     wq_sb[:, et * 128:(et + 1) * 128],
                                     xT[:, et * N + c * CHUNK:
                                        et * N + (c + 1) * CHUNK],
                                     start=(et == 0), stop=(et == 7))
                    if et % 2 == 1:
                        yield
                kvps = psP.tile([128, 512], f32, tag="ps_small")
                for et in range(8):
                    nc.tensor.matmul(kvps[:],
                                     wkv_sb[:, et * 128:(et + 1) * 128],
                                     xT[:, et * N + c * CHUNK:
                                        et * N + (c + 1) * CHUNK],
                                     start=(et == 0), stop=(et == 7))
                    if et % 2 == 1:
                        yield
                yield from rope_epilogue(c, tiles, qps[:], kvps[:])

            # Schraudolph exp in one DVE tensor_scalar: int16(x*s + b) whose
            # bits ARE bf16 exp(x*SCALE). Max rel err ~3.5%.
            SCH_S = float(SCALE * 128.0 * np.log2(np.e))
            SCH_B = float(127 * 128 - 6)

            def proj_qkv_wide(b, h, tiles, xT):
                """Prologue-only: N=1024 matmuls into the (idle) psS banks,
                half-batch h of 1024 tokens. Dense PE stream keeps HAM warm."""
                qT, kdup, v_sb = tiles
                hs = slice(h * 1024, (h + 1) * 1024)
                qps = psS.tile([128, 1024], f32, tag="ps_big")
                for et in range(8):
                    for hf in range(2):
                        nc.tensor.matmul(qps[:, hf * 512:(hf + 1) * 512],
                                         wq_sb[:, et * 128:(et + 1) * 128],
                                         xT[:, et * N + h * 1024 + hf * 512:
                                            et * N + h * 1024 + (hf + 1) * 512],
                                         start=(et == 0), stop=(et == 7))
                kvps = psS.tile([128, 1024], f32, tag="ps_big")
                for et in range(8):
                    for hf in range(2):
                        nc.tensor.matmul(kvps[:, hf * 512:(hf + 1) * 512],
                                         wkv_sb[:, et * 128:(et + 1) * 128],
                                         xT[:, et * N + h * 1024 + hf * 512:
                                            et * N + h * 1024 + (hf + 1) * 512],
                                         start=(et == 0), stop=(et == 7))
                for half in range(2):
                    c = h * 2 + half
                    cs512 = slice(half * 512, (half + 1) * 512)
                    for _ in rope_epilogue(c, tiles, qps[:, cs512],
                                           kvps[:, cs512]):
                        pass

            def scores_step(b, qb, kt, tiles, ndve):
                qT, kdup, v_sb = tiles
                qs = slice(qb * QB, (qb + 1) * QB)
                sps = psS.tile([128, 1024], f32, tag="ps_big")
                nc.tensor.matmul(sps[:, 0:512],
                                 kdup[0:64, kt * 128:(kt + 1) * 128],
                                 qT[0:64, qs], start=True, stop=True)
                nc.tensor.matmul(sps[:, 512:1024],
                                 kdup[64:128, kt * 128:(kt + 1) * 128],
                                 qT[64:128, qs], start=True, stop=True)
                e_sb = expp.tile([128, 1024], bf16, tag="e")
                if ndve and kt in range(5, 5 + 3 * ndve, 3):
                    nc.vector.tensor_scalar(
                        e_sb[:].bitcast(dt.int16), sps[:],
                        SCH_S, SCH_B, op0=OP.mult, op1=OP.add)
                else:
                    nc.scalar.activation(e_sb[:], sps[:], AF.Exp, scale=SCALE)
                if debug and b == 0 and qb == 0 and kt == 0:
                    nc.sync.dma_start(dbg_e[:], e_sb[:])
                return e_sb

            def attn_v_step(ops_t, tiles, e_sb, kt):
                # v padded to 128 cols (ones col 64, zeros 65:127) so the
                # weight load gets FWL; rows 65:127 of ops_t accumulate zeros
                v_sb = tiles[2]
                nc.tensor.matmul(ops_t[:, 0:512],
                                 v_sb[:, kt * 128:(kt + 1) * 128],
                                 e_sb[:, 0:512],
                                 start=(kt == 0), stop=(kt == NKT - 1))
                nc.tensor.matmul(ops_t[:, 512:1024],
                                 v_sb[:, kt * 128:(kt + 1) * 128],
                                 e_sb[:, 512:1024],
                                 start=(kt == 0), stop=(kt == NKT - 1))

            def attn_out_norm(ops_t, dbg=False):
                den = smp.tile([1, 1024], f32, tag="den")
                nc.vector.tensor_copy(den[:], ops_t[64:65, :])
                # free the PSUM accumulator early
                ou = smp.tile([64, 1024], f32, tag="ou")
                nc.vector.tensor_copy(ou[:], ops_t[0:64, :])
                rec = smp.tile([1, 1024], f32, tag="rq")
                nc.vector.reciprocal_approx_fast(rec[:], den[:])
                if dbg:
                    nc.sync.dma_start(dbg_ou[:], ou[:])
                    nc.sync.dma_start(dbg_rec[:], rec[:])
                rb = smp.tile([64, 1024], f32, tag="rb")
                nc.gpsimd.partition_broadcast(rb[:], rec[:])
                oT = osbp.tile([128, QB], bf16, tag="o")
                nc.vector.tensor_tensor(oT[0:64, :], ou[:, 0:512],
                                        rb[:, 0:512], op=OP.mult)
                o1 = osbp.tile([64, QB], bf16, tag="o1")
                nc.vector.tensor_tensor(o1[:], ou[:, 512:1024],
                                        rb[:, 512:1024], op=OP.mult)
                nc.sync.dma_start(oT[64:128, :], o1[:])
                return oT

            def attn_out_proj(b, qb, oT, which):
                for n in which:
                    ts, eh = n // 2, n % 2
                    po = psP.tile([128, 512], f32, tag="ps_small")
                    nc.tensor.matmul(po[:],
                                     oT[:, ts * 128:(ts + 1) * 128],
                                     wo_sb[:, eh * 512:(eh + 1) * 512],
                                     start=True, stop=True)
                    ob = osbp.tile([128, 512], bf16, tag="ob")
                    if n < 2:
                        nc.scalar.copy(ob[:], po[:])
                    else:
                        nc.vector.tensor_copy(ob[:], po[:])
                    r0 = b * N + qb * QB + ts * 128
                    nc.sync.dma_start(
                        out_d[r0:r0 + 128, eh * 512:(eh + 1) * 512], ob[:])

            def batch_tiles(b):
                qT = batchp.tile([128, N], bf16, tag="qT")
                kdup = batchp.tile([128, N], bf16, tag="kdup")
                v_sb = batchp.tile([128, NKT * 128], bf16, tag="v")
                v3 = v_sb[:].rearrange("p (kt c) -> p kt c", c=128)
                nc.vector.memset(v3[:, :, 65:128], 0.0)
                nc.vector.memset(v3[:, :, 64:65], 1.0)
                return (qT, kdup, v_sb)

            # ---- flattened global pipeline over all B*NQB*NKT kt-steps ----
            # Per step s: scores+exp for step s, attnV for step s-LAG (6us of
            # slack behind exp), proj-filler for the next batch, and the
            # normalize / out-projection of finished q-blocks as deferred
            # tasks. No q-block boundary ever serializes the PE FIFO.
            LAG = 6
            S = B * NQB * NKT  # 256
            tiles_arr = {0: batch_tiles(0)}
            xT_arr = {0: load_xt(0, wide=True)}
            # prologue: only tokens 0:1024 (chunks 0,1) of batch 0 projected
            # upfront; chunks 2,3 stream as fillers during the first steps
            proj_qkv_wide(0, 0, tiles_arr[0], xT_arr[0])
            xT_arr[1] = load_xt(1)

            es_all = {}
            ops_map = {}
            oT_map = {}
            pending_pre = []   # DVE/gpsimd-only tasks: run before scores
            pending_post = []  # tasks with PE matmuls: run after attnV
            seqn = [0]

            def defer(q, due, fn):
                q.append((due, seqn[0], fn))
                seqn[0] += 1

            def run_due(q, s):
                for due, _, fn in sorted([p for p in q if p[0] <= s],
                                         key=lambda p: (p[0], p[1])):
                    fn()
                q[:] = [p for p in q if p[0] > s]

            from collections import deque
            filler_q = deque()
            filler_q.append(proj_qkv(0, 2, tiles_arr[0], xT_arr[0]))
            filler_q.append(proj_qkv(0, 3, tiles_arr[0], xT_arr[0]))
            s = 0
            while s < S + LAG or pending_pre or pending_post:
                run_due(pending_pre, s)
                if s < S:
                    b, r = divmod(s, NQB * NKT)
                    qb, kt = divmod(r, NKT)
                    if r == 0:
                        if b + 1 < B:
                            tiles_arr[b + 1] = batch_tiles(b + 1)
                        if b + 2 < B:
                            xT_arr[b + 2] = load_xt(b + 2)
                    ndve = 3 if b == B - 1 else 0
                    es_all[s] = scores_step(b, qb, kt, tiles_arr[b], ndve)
                    if debug and s == NKT + 1:
                        qT0, kdup0, v0 = tiles_arr[0]
                        nc.sync.dma_start(dbg_qT[:], qT0[:])
                        nc.sync.dma_start(dbg_kdup[:], kdup0[:])
                        nc.sync.dma_start(dbg_v[:], v0[:])
                    # proj filler: next batch's chunk qb queued at this
                    # q-block (shifted one q-block later for batch 1 because
                    # batch 0's own chunks 2,3 stream during qb0)
                    if b + 1 < B and kt == 0 and not (b == 0 and qb == 0):
                        c = qb - 1 if b == 0 else qb
                        filler_q.append(proj_qkv(
                            b + 1, c, tiles_arr[b + 1], xT_arr[b + 1]))
                        if b == 0 and qb == 3:
                            filler_q.append(proj_qkv(
                                1, 3, tiles_arr[1], xT_arr[1]))
                    budget = 5 if s < 12 else 3
                    while budget > 0 and filler_q:
                        try:
                            next(filler_q[0])
                            budget -= 1
                        except StopIteration:
                            filler_q.popleft()
                s2 = s - LAG
                if 0 <= s2 < S:
                    b2, r2 = divmod(s2, NQB * NKT)
                    qb2, kt2 = divmod(r2, NKT)
                    if kt2 == 0:
                        ops_map[(b2, qb2)] = psO.tile(
                            [128, 1024], f32, tag="ps_o",
                            name=f"ops_{b2}_{qb2}")
                    attn_v_step(ops_map[(b2, qb2)], tiles_arr[b2],
                                es_all.pop(s2), kt2)
                    if kt2 == NKT - 1:
                        def mk_norm(b2=b2, qb2=qb2):
                            def go():
                                oT_map[(b2, qb2)] = attn_out_norm(
                                    ops_map.pop((b2, qb2)),
                                    dbg=debug and b2 == 0 and qb2 == 0)
                            return go
                        defer(pending_pre, s + 1, mk_norm())
                        for gi in range(4):
                            def mk_po(b2=b2, qb2=qb2, gi=gi):
                                def go():
                                    attn_out_proj(
                                        b2, qb2, oT_map[(b2, qb2)],
                                        range(gi * 2, gi * 2 + 2))
                                    if gi == 3:
                                        oT_map.pop((b2, qb2))
                                return go
                            defer(pending_post, s + 8 + gi, mk_po())
                run_due(pending_post, s)
                s += 1

    nc.compile()
    _BUILT[key] = nc
    return nc


def _make_in_maps(x, Wq, Wk, Wv, Wo):
    import ml_dtypes
    bf16 = ml_dtypes.bfloat16
    cos_t, sin_t = _rope_tables()
    xt = np.ascontiguousarray(
        x.reshape(T, DIM).T.astype(bf16))  # [DIM, T]
    in_maps = []
    for d in range(N_CORES):
        # weight layouts: [128 part = x-dim chunk et, 8*128 cols], the et-th
        # 128-col block holds rows et*128:(et+1)*128 of the weight
        wq_d = np.concatenate(
            [Wq[et * 128:(et + 1) * 128, d * 128:(d + 1) * 128]
             for et in range(8)], axis=1).astype(bf16)
        wkv_cols = np.concatenate(
            [Wk[:, d * 64:(d + 1) * 64], Wv[:, d * 64:(d + 1) * 64]], axis=1)
        wkv_d = np.concatenate(
            [wkv_cols[et * 128:(et + 1) * 128, :] for et in range(8)],
            axis=1).astype(bf16)
        wo_d = np.ascontiguousarray(Wo[d * 128:(d + 1) * 128, :]).astype(bf16)
        in_maps.append({
            "xt": xt, "wq": np.ascontiguousarray(wq_d),
            "wkv": np.ascontiguousarray(wkv_d), "wo": wo_d,
            "cos_t": cos_t, "sin_t": sin_t,
        })
    return in_maps


def _run(in_maps, trace=False, trace_kwargs=None, debug=False):
    _ensure_path()
    from concourse.bass_utils import run_bass_kernel_spmd
    nc = _build(debug=debug)
    return run_bass_kernel_spmd(nc, in_maps, list(range(N_CORES)), trace=trace,
                                **(trace_kwargs or {}))


def kernel(x, Wq, Wk, Wv, Wo, bo):
    x = np.asarray(x, dtype=np.float32)
    in_maps = _make_in_maps(np.ascontiguousarray(x.reshape(B, N, DIM)),
                            np.asarray(Wq, np.float32), np.asarray(Wk, np.float32),
                            np.asarray(Wv, np.float32), np.asarray(Wo, np.float32))
    res = _run(in_maps)
    acc = np.zeros((T, DIM), dtype=np.float32)
    for d in range(N_CORES):
        acc += np.asarray(res.results[d]["out"], dtype=np.float32)
    acc += np.asarray(bo, np.float32)[None, :]
    return acc.reshape(B, N, DIM)
